# revision 2
# baseline (speedup 1.0000x reference)
"""Trainium2 Bass kernel for nn_DON_cnn_79216376807825 (histogram_binning).

Strategy (8 NeuronCores, data-parallel over points):
  - The dominant cost is two 4-layer MLPs (3->256->256->256->256, tanh) applied
    to all 262144 points, each followed by a max-reduction over points.
    Points are sharded 32768/core; each core computes its partial max of the
    final (pre-bias) layer output; host reduces over cores and adds the final
    bias (max(h@W + b) == max(h@W) + b).
  - On-chip layout: features on partitions, points on the free dim.  Weights
    are the stationary matmul operand (lhsT = W as stored, [K_in, M_out]);
    activations stream as the moving operand.  float32r matmuls (1 cyc/row,
    ~2e-4 rel err).  tanh+bias on the scalar engine reading PSUM, writing
    SBUF.  Final-layer PSUM is max-reduced on the vector engine.
  - The tiny patch part (gather of ~260 points in bin 995, tr-MLP, concat,
    o-MLP) runs on host in fp32 numpy - it is <0.03% of the FLOPs.
"""

import sys

if "/opt/trn_rl_repo" not in sys.path:
    sys.path.insert(0, "/opt/trn_rl_repo")

import numpy as np

import concourse.bass as bass  # noqa: F401  (engine registration side effects)
import concourse.mybir as mybir
from concourse import bacc, tile
from concourse.bass_utils import run_bass_kernel_spmd

N_CORES = 8
N_PTS = 262144
P = N_PTS // N_CORES          # 32768 points per core
T = 512                       # points per macro-tile (psum tile = T f32 cols)
NT = P // T
H = 256
MNK = 10
PATCH_ID = 995

F32 = mybir.dt.float32
F32R = mybir.dt.float32r
AF = mybir.ActivationFunctionType
AX = mybir.AxisListType

_CACHE: dict = {}


def _build():
    nc = bacc.Bacc("TRN2", target_bir_lowering=False, debug=False,
                   num_devices=N_CORES)
    xt_d = nc.dram_tensor("xt", [3, P], F32R, kind="ExternalInput").ap()
    w0_d = nc.dram_tensor("w0", [3, 512], F32R, kind="ExternalInput").ap()
    wk_d = nc.dram_tensor("wk", [128, 3072], F32R, kind="ExternalInput").ap()
    bs_d = nc.dram_tensor("bs", [128, 12], F32, kind="ExternalInput").ap()
    om_d = nc.dram_tensor("omax", [128, 4], F32, kind="ExternalOutput").ap()

    ncb = T // 512  # 512-col blocks per tile (matmul moving-operand limit)

    with tile.TileContext(nc) as tc:
        with tc.tile_pool(name="const", bufs=1) as cpool, \
             tc.tile_pool(name="xtp", bufs=4) as xpool, \
             tc.tile_pool(name="act", bufs=8) as apool, \
             tc.tile_pool(name="ps", bufs=8, space="PSUM") as pspool, \
             tc.tile_pool(name="red", bufs=1) as rpool:
            w0_s = cpool.tile([3, 512], F32R, tag="w0")
            wk_s = cpool.tile([128, 3072], F32R, tag="wk")
            bs_s = cpool.tile([128, 12], F32, tag="bs")
            nc.sync.dma_start(w0_s[:], w0_d[:])
            nc.sync.dma_start(wk_s[:], wk_d[:])
            nc.sync.dma_start(bs_s[:], bs_d[:])
            # per-(chunk, tile) reduced maxima; final pass reduces over tiles
            rm = rpool.tile([128, 4, NT], F32, tag="rm")
            om_s = rpool.tile([128, 4], F32, tag="om")

            for t in range(NT):
                xt_t = xpool.tile([3, T], F32R, tag="xt")
                nc.sync.dma_start(xt_t[:], xt_d[:, t * T:(t + 1) * T])
                for m in range(2):          # 0 = tb (global), 1 = br (local)
                    # ---- layer 0: (3 -> 256) ----
                    ps0 = [pspool.tile([128, T], F32, tag="ps", name=f"ps0_{t}_{m}_{jj}")
                           for jj in range(2)]
                    for j in range(2):
                        for cb in range(ncb):
                            nc.tensor.matmul(
                                ps0[j][:, cb * 512:(cb + 1) * 512],
                                w0_s[:, m * 256 + j * 128:m * 256 + (j + 1) * 128],
                                xt_t[:, cb * 512:(cb + 1) * 512],
                                start=True, stop=True)
                    a0 = apool.tile([128, 2, T], F32R, tag="a")
                    for j in range(2):
                        nc.scalar.activation(
                            a0[:, j, :], ps0[j][:], AF.Tanh,
                            bias=bs_s[:, m * 6 + j:m * 6 + j + 1], scale=1.0)
                    prev = a0
                    # ---- layers 1..3: (256 -> 256) ----
                    for l in (1, 2, 3):
                        psl = [pspool.tile([128, T], F32, tag="ps", name=f"ps{l}_{t}_{m}_{jj}")
                               for jj in range(2)]
                        for j in range(2):
                            for cb in range(ncb):
                                for k in range(2):
                                    b = ((m * 3 + (l - 1)) * 2 + k) * 2 + j
                                    nc.tensor.matmul(
                                        psl[j][:, cb * 512:(cb + 1) * 512],
                                        wk_s[:, b * 128:(b + 1) * 128],
                                        prev[:, k, cb * 512:(cb + 1) * 512],
                                        start=(k == 0), stop=(k == 1))
                        if l < 3:
                            al = apool.tile([128, 2, T], F32R, tag="a")
                            for j in range(2):
                                col = m * 6 + l * 2 + j
                                nc.scalar.activation(
                                    al[:, j, :], psl[j][:], AF.Tanh,
                                    bias=bs_s[:, col:col + 1], scale=1.0)
                            prev = al
                        else:
                            for j in range(2):
                                nc.vector.reduce_max(
                                    rm[:, m * 2 + j, t:t + 1], psl[j][:],
                                    axis=AX.X)
            for c in range(4):
                nc.vector.reduce_max(om_s[:, c:c + 1], rm[:, c, :], axis=AX.X)
            nc.sync.dma_start(om_d[:], om_s[:])
    nc.compile()
    return nc


def _get_nc():
    if "nc" not in _CACHE:
        _CACHE["nc"] = _build()
    return _CACHE["nc"]


def _pack_weights(g):
    """g maps name -> np.ndarray for the tb_*/br_* weights."""
    w0 = np.concatenate([g["tb_w0"], g["br_w0"]], axis=1).astype(np.float32)
    blocks = []
    for pre in ("tb", "br"):
        for l in (1, 2, 3):
            W = g[f"{pre}_w{l}"]
            for k in range(2):
                for j in range(2):
                    blocks.append(W[k * 128:(k + 1) * 128,
                                    j * 128:(j + 1) * 128])
    wk = np.ascontiguousarray(np.concatenate(blocks, axis=1), dtype=np.float32)
    bs = np.zeros((128, 12), np.float32)
    for mi, pre in enumerate(("tb", "br")):
        for l in range(3):
            bvec = g[f"{pre}_b{l}"]
            for j in range(2):
                bs[:, mi * 6 + l * 2 + j] = bvec[j * 128:(j + 1) * 128]
    return w0, wk, bs


def _run_device(x, g, trace=False):
    """Returns (tb_max, br_max) pre-bias maxima of shape (256,) each, plus
    the BassKernelResults (for profiling)."""
    w0, wk, bs = _pack_weights(g)
    in_maps = []
    for c in range(N_CORES):
        xt = np.ascontiguousarray(x[c * P:(c + 1) * P].T, dtype=np.float32)
        in_maps.append({"xt": xt, "w0": w0, "wk": wk, "bs": bs})
    res = run_bass_kernel_spmd(_get_nc(), in_maps, list(range(N_CORES)),
                               trace=trace)
    oms = np.stack([r["omax"] for r in res.results])     # (8, 128, 4)
    om = oms.max(axis=0)                                 # (128, 4)
    tb_max = np.concatenate([om[:, 0], om[:, 1]])        # (256,)
    br_max = np.concatenate([om[:, 2], om[:, 3]])
    return tb_max, br_max, res


def _mlp_np(h, layers):
    for w, b in layers[:-1]:
        h = np.tanh(h @ w + b)
    w, b = layers[-1]
    return h @ w + b


def kernel(x, y,
           tb_w0, tb_b0, tb_w1, tb_b1, tb_w2, tb_b2, tb_w3, tb_b3,
           br_w0, br_b0, br_w1, br_b1, br_w2, br_b2, br_w3, br_b3,
           tr_w0, tr_b0, tr_w1, tr_b1, tr_w2, tr_b2, tr_w3, tr_b3,
           o_w0, o_b0, o_w1, o_b1, o_w2, o_b2, _trace=False):
    x = np.asarray(x, np.float32)
    y = np.asarray(y, np.float32)
    g = {k: np.asarray(v, np.float32) for k, v in dict(
        tb_w0=tb_w0, tb_w1=tb_w1, tb_w2=tb_w2, tb_w3=tb_w3,
        br_w0=br_w0, br_w1=br_w1, br_w2=br_w2, br_w3=br_w3,
        tb_b0=tb_b0, tb_b1=tb_b1, tb_b2=tb_b2,
        br_b0=br_b0, br_b1=br_b1, br_b2=br_b2,
    ).items()}

    tb_pre, br_pre, res = _run_device(x, g, trace=_trace)
    _CACHE["last_results"] = res
    global_param = tb_pre + np.asarray(tb_b3, np.float32)   # (256,)
    local_param = br_pre + np.asarray(br_b3, np.float32)

    # patch gather (host): points whose bin id == PATCH_ID
    c = np.clip(np.floor(x * float(MNK)).astype(np.int64), 0, MNK - 1)
    pid = c[:, 0] * (MNK * MNK) + c[:, 1] * MNK + c[:, 2]
    idx = np.nonzero(pid == PATCH_ID)[0]
    x_patch = x[idx]
    gt_patch = y[idx]

    tr = [(np.asarray(tr_w0, np.float32), np.asarray(tr_b0, np.float32)),
          (np.asarray(tr_w1, np.float32), np.asarray(tr_b1, np.float32)),
          (np.asarray(tr_w2, np.float32), np.asarray(tr_b2, np.float32)),
          (np.asarray(tr_w3, np.float32), np.asarray(tr_b3, np.float32))]
    o = [(np.asarray(o_w0, np.float32), np.asarray(o_b0, np.float32)),
         (np.asarray(o_w1, np.float32), np.asarray(o_b1, np.float32)),
         (np.asarray(o_w2, np.float32), np.asarray(o_b2, np.float32))]

    local_coord = _mlp_np(x_patch, tr)                      # (MM, 256)
    mm = local_coord.shape[0]
    feat = np.concatenate([
        local_coord,
        np.broadcast_to(local_param, (mm, local_param.shape[0])),
        np.broadcast_to(global_param, (mm, global_param.shape[0])),
    ], axis=-1).astype(np.float32)
    pred_patch = _mlp_np(feat, o).astype(np.float32)
    return pred_patch, gt_patch


# revision 3
# speedup vs baseline: 1.4750x; 1.4750x over previous
"""Trainium2 Bass kernel for nn_DON_cnn_79216376807825 (histogram_binning).

Strategy (8 NeuronCores, data-parallel over points):
  - The dominant cost is two 4-layer MLPs (3->256->256->256->256, tanh) applied
    to all 262144 points, each followed by a max-reduction over points.
    Points are sharded 32768/core; each core computes its partial max of the
    final (pre-bias) layer output; host reduces over cores and adds the final
    bias (max(h@W + b) == max(h@W) + b).
  - On-chip layout: features on partitions, points on the free dim.  Weights
    are the stationary matmul operand (lhsT = W as stored, [K_in, M_out]);
    activations stream as the moving operand.  float32r matmuls (1 cyc/row,
    ~2e-4 rel err).  tanh+bias on the scalar engine reading PSUM, writing
    SBUF.  Final-layer PSUM is max-reduced on the vector engine.
  - The tiny patch part (gather of ~260 points in bin 995, tr-MLP, concat,
    o-MLP) runs on host in fp32 numpy - it is <0.03% of the FLOPs.
"""

import sys

if "/opt/trn_rl_repo" not in sys.path:
    sys.path.insert(0, "/opt/trn_rl_repo")

import numpy as np

import concourse.bass as bass  # noqa: F401  (engine registration side effects)
import concourse.mybir as mybir
from concourse import bacc, tile
from concourse.bass_utils import run_bass_kernel_spmd

N_CORES = 8
N_PTS = 262144
P = N_PTS // N_CORES          # 32768 points per core
T = 1024                      # points per macro-tile (psum tile = T f32 cols)
NT = P // T
H = 256
MNK = 10
PATCH_ID = 995

F32 = mybir.dt.float32
F32R = mybir.dt.float32r
F16 = mybir.dt.float16
DT = F16                      # matmul operand dtype (fp16: 1 cyc/row, ~4e-4)
NPDT = np.float16
AF = mybir.ActivationFunctionType
AX = mybir.AxisListType

_CACHE: dict = {}


def _build():
    nc = bacc.Bacc("TRN2", target_bir_lowering=False, debug=False,
                   num_devices=N_CORES)
    xt_d = nc.dram_tensor("xt", [3, P], DT, kind="ExternalInput").ap()
    w0_d = nc.dram_tensor("w0", [3, 512], DT, kind="ExternalInput").ap()
    wk_d = nc.dram_tensor("wk", [128, 3072], DT, kind="ExternalInput").ap()
    bs_d = nc.dram_tensor("bs", [128, 12], F32, kind="ExternalInput").ap()
    om_d = nc.dram_tensor("omax", [128, 4], F32, kind="ExternalOutput").ap()

    ncb = T // 512  # 512-col blocks per tile (matmul moving-operand limit)

    with tile.TileContext(nc) as tc:
        with tc.tile_pool(name="const", bufs=1) as cpool, \
             tc.tile_pool(name="xtp", bufs=4) as xpool, \
             tc.tile_pool(name="act", bufs=8) as apool, \
             tc.tile_pool(name="ps", bufs=4, space="PSUM") as pspool, \
             tc.tile_pool(name="red", bufs=1) as rpool:
            w0_s = cpool.tile([3, 512], DT, tag="w0")
            wk_s = cpool.tile([128, 3072], DT, tag="wk")
            bs_s = cpool.tile([128, 12], F32, tag="bs")
            nc.sync.dma_start(w0_s[:], w0_d[:])
            nc.sync.dma_start(wk_s[:], wk_d[:])
            nc.sync.dma_start(bs_s[:], bs_d[:])
            # per-(chunk, tile) reduced maxima; final pass reduces over tiles
            rm = rpool.tile([128, 4, NT], F32, tag="rm")
            om_s = rpool.tile([128, 4], F32, tag="om")

            for t in range(NT):
                xt_t = xpool.tile([3, T], DT, tag="xt")
                nc.sync.dma_start(xt_t[:], xt_d[:, t * T:(t + 1) * T])
                for m in range(2):          # 0 = tb (global), 1 = br (local)
                    # ---- layer 0: (3 -> 256) ----
                    ps0 = [pspool.tile([128, T], F32, tag="ps", name=f"ps0_{t}_{m}_{jj}")
                           for jj in range(2)]
                    for j in range(2):
                        for cb in range(ncb):
                            nc.tensor.matmul(
                                ps0[j][:, cb * 512:(cb + 1) * 512],
                                w0_s[:, m * 256 + j * 128:m * 256 + (j + 1) * 128],
                                xt_t[:, cb * 512:(cb + 1) * 512],
                                start=True, stop=True)
                    a0 = apool.tile([128, 2, T], DT, tag="a")
                    for j in range(2):
                        nc.scalar.activation(
                            a0[:, j, :], ps0[j][:], AF.Tanh,
                            bias=bs_s[:, m * 6 + j:m * 6 + j + 1], scale=1.0)
                    prev = a0
                    # ---- layers 1..3: (256 -> 256) ----
                    for l in (1, 2, 3):
                        psl = [pspool.tile([128, T], F32, tag="ps", name=f"ps{l}_{t}_{m}_{jj}")
                               for jj in range(2)]
                        for j in range(2):
                            for k in range(2):
                                b = ((m * 3 + (l - 1)) * 2 + k) * 2 + j
                                for cb in range(ncb):
                                    nc.tensor.matmul(
                                        psl[j][:, cb * 512:(cb + 1) * 512],
                                        wk_s[:, b * 128:(b + 1) * 128],
                                        prev[:, k, cb * 512:(cb + 1) * 512],
                                        start=(k == 0), stop=(k == 1))
                        if l < 3:
                            al = apool.tile([128, 2, T], DT, tag="a")
                            for j in range(2):
                                col = m * 6 + l * 2 + j
                                nc.scalar.activation(
                                    al[:, j, :], psl[j][:], AF.Tanh,
                                    bias=bs_s[:, col:col + 1], scale=1.0)
                            prev = al
                        else:
                            for j in range(2):
                                nc.vector.reduce_max(
                                    rm[:, m * 2 + j, t:t + 1], psl[j][:],
                                    axis=AX.X)
            for c in range(4):
                nc.vector.reduce_max(om_s[:, c:c + 1], rm[:, c, :], axis=AX.X)
            nc.sync.dma_start(om_d[:], om_s[:])
    nc.compile()
    return nc


def _get_nc():
    if "nc" not in _CACHE:
        _CACHE["nc"] = _build()
    return _CACHE["nc"]


def _pack_weights(g):
    """g maps name -> np.ndarray for the tb_*/br_* weights."""
    w0 = np.concatenate([g["tb_w0"], g["br_w0"]], axis=1).astype(NPDT)
    blocks = []
    for pre in ("tb", "br"):
        for l in (1, 2, 3):
            W = g[f"{pre}_w{l}"]
            for k in range(2):
                for j in range(2):
                    blocks.append(W[k * 128:(k + 1) * 128,
                                    j * 128:(j + 1) * 128])
    wk = np.ascontiguousarray(np.concatenate(blocks, axis=1), dtype=NPDT)
    bs = np.zeros((128, 12), np.float32)
    for mi, pre in enumerate(("tb", "br")):
        for l in range(3):
            bvec = g[f"{pre}_b{l}"]
            for j in range(2):
                bs[:, mi * 6 + l * 2 + j] = bvec[j * 128:(j + 1) * 128]
    return w0, wk, bs


def _run_device(x, g, trace=False):
    """Returns (tb_max, br_max) pre-bias maxima of shape (256,) each, plus
    the BassKernelResults (for profiling)."""
    w0, wk, bs = _pack_weights(g)
    in_maps = []
    for c in range(N_CORES):
        xt = np.ascontiguousarray(x[c * P:(c + 1) * P].T, dtype=NPDT)
        in_maps.append({"xt": xt, "w0": w0, "wk": wk, "bs": bs})
    res = run_bass_kernel_spmd(_get_nc(), in_maps, list(range(N_CORES)),
                               trace=trace)
    oms = np.stack([r["omax"] for r in res.results])     # (8, 128, 4)
    om = oms.max(axis=0)                                 # (128, 4)
    tb_max = np.concatenate([om[:, 0], om[:, 1]])        # (256,)
    br_max = np.concatenate([om[:, 2], om[:, 3]])
    return tb_max, br_max, res


def _mlp_np(h, layers):
    for w, b in layers[:-1]:
        h = np.tanh(h @ w + b)
    w, b = layers[-1]
    return h @ w + b


def kernel(x, y,
           tb_w0, tb_b0, tb_w1, tb_b1, tb_w2, tb_b2, tb_w3, tb_b3,
           br_w0, br_b0, br_w1, br_b1, br_w2, br_b2, br_w3, br_b3,
           tr_w0, tr_b0, tr_w1, tr_b1, tr_w2, tr_b2, tr_w3, tr_b3,
           o_w0, o_b0, o_w1, o_b1, o_w2, o_b2, _trace=False):
    x = np.asarray(x, np.float32)
    y = np.asarray(y, np.float32)
    g = {k: np.asarray(v, np.float32) for k, v in dict(
        tb_w0=tb_w0, tb_w1=tb_w1, tb_w2=tb_w2, tb_w3=tb_w3,
        br_w0=br_w0, br_w1=br_w1, br_w2=br_w2, br_w3=br_w3,
        tb_b0=tb_b0, tb_b1=tb_b1, tb_b2=tb_b2,
        br_b0=br_b0, br_b1=br_b1, br_b2=br_b2,
    ).items()}

    tb_pre, br_pre, res = _run_device(x, g, trace=_trace)
    _CACHE["last_results"] = res
    global_param = tb_pre + np.asarray(tb_b3, np.float32)   # (256,)
    local_param = br_pre + np.asarray(br_b3, np.float32)

    # patch gather (host): points whose bin id == PATCH_ID
    c = np.clip(np.floor(x * float(MNK)).astype(np.int64), 0, MNK - 1)
    pid = c[:, 0] * (MNK * MNK) + c[:, 1] * MNK + c[:, 2]
    idx = np.nonzero(pid == PATCH_ID)[0]
    x_patch = x[idx]
    gt_patch = y[idx]

    tr = [(np.asarray(tr_w0, np.float32), np.asarray(tr_b0, np.float32)),
          (np.asarray(tr_w1, np.float32), np.asarray(tr_b1, np.float32)),
          (np.asarray(tr_w2, np.float32), np.asarray(tr_b2, np.float32)),
          (np.asarray(tr_w3, np.float32), np.asarray(tr_b3, np.float32))]
    o = [(np.asarray(o_w0, np.float32), np.asarray(o_b0, np.float32)),
         (np.asarray(o_w1, np.float32), np.asarray(o_b1, np.float32)),
         (np.asarray(o_w2, np.float32), np.asarray(o_b2, np.float32))]

    local_coord = _mlp_np(x_patch, tr)                      # (MM, 256)
    mm = local_coord.shape[0]
    feat = np.concatenate([
        local_coord,
        np.broadcast_to(local_param, (mm, local_param.shape[0])),
        np.broadcast_to(global_param, (mm, global_param.shape[0])),
    ], axis=-1).astype(np.float32)
    pred_patch = _mlp_np(feat, o).astype(np.float32)
    return pred_patch, gt_patch


# revision 4
# speedup vs baseline: 1.9811x; 1.3432x over previous
"""Trainium2 Bass kernel for nn_DON_cnn_79216376807825 (histogram_binning).

Strategy (8 NeuronCores, data-parallel over points):
  - The dominant cost is two 4-layer MLPs (3->256->256->256->256, tanh) applied
    to all 262144 points, each followed by a max-reduction over points.
    Points are sharded 32768/core; each core computes its partial max of the
    final (pre-bias) layer output; host reduces over cores and adds the final
    bias (max(h@W + b) == max(h@W) + b).
  - On-chip layout: features on partitions, points on the free dim.  Weights
    are the stationary matmul operand (lhsT = W as stored, [K_in, M_out]);
    activations stream as the moving operand.  float32r matmuls (1 cyc/row,
    ~2e-4 rel err).  tanh+bias on the scalar engine reading PSUM, writing
    SBUF.  Final-layer PSUM is max-reduced on the vector engine.
  - The tiny patch part (gather of ~260 points in bin 995, tr-MLP, concat,
    o-MLP) runs on host in fp32 numpy - it is <0.03% of the FLOPs.
"""

import sys

if "/opt/trn_rl_repo" not in sys.path:
    sys.path.insert(0, "/opt/trn_rl_repo")

import numpy as np

import concourse.bass as bass  # noqa: F401  (engine registration side effects)
import concourse.mybir as mybir
from concourse import bacc, tile
from concourse.bass_utils import run_bass_kernel_spmd

N_CORES = 8
N_PTS = 262144
P = N_PTS // N_CORES          # 32768 points per core
T = 1024                      # points per macro-tile (psum tile = T f32 cols)
NT = P // T
H = 256
MNK = 10
PATCH_ID = 995

F32 = mybir.dt.float32
F32R = mybir.dt.float32r
F16 = mybir.dt.float16
DT = F16                      # matmul operand dtype (fp16: 1 cyc/row, ~4e-4)
NPDT = np.float16
AF = mybir.ActivationFunctionType
AX = mybir.AxisListType

_CACHE: dict = {}


def _build():
    nc = bacc.Bacc("TRN2", target_bir_lowering=False, debug=False,
                   num_devices=N_CORES)
    xt_d = nc.dram_tensor("xt", [3, P], DT, kind="ExternalInput").ap()
    w0_d = nc.dram_tensor("w0", [3, 512], DT, kind="ExternalInput").ap()
    wk_d = nc.dram_tensor("wk", [128, 3072], DT, kind="ExternalInput").ap()
    bs_d = nc.dram_tensor("bs", [128, 12], F32, kind="ExternalInput").ap()
    om_d = nc.dram_tensor("omax", [128, 4], F32, kind="ExternalOutput").ap()

    ncb = T // 512  # 512-col blocks per tile (matmul moving-operand limit)

    with tile.TileContext(nc) as tc:
        with tc.tile_pool(name="const", bufs=1) as cpool, \
             tc.tile_pool(name="xtp", bufs=4) as xpool, \
             tc.tile_pool(name="act", bufs=8) as apool, \
             tc.tile_pool(name="ps", bufs=4, space="PSUM") as pspool, \
             tc.tile_pool(name="red", bufs=1) as rpool:
            w0_s = cpool.tile([3, 512], DT, tag="w0")
            wk_s = cpool.tile([128, 3072], DT, tag="wk")
            bs_s = cpool.tile([128, 12], F32, tag="bs")
            nc.sync.dma_start(w0_s[:], w0_d[:])
            nc.sync.dma_start(wk_s[:], wk_d[:])
            nc.sync.dma_start(bs_s[:], bs_d[:])
            # per-(chunk, tile) reduced maxima; final pass reduces over tiles
            rm = rpool.tile([128, 4, NT], F32, tag="rm")
            om_s = rpool.tile([128, 4], F32, tag="om")

            for t in range(NT):
                xt_t = xpool.tile([3, T], DT, tag="xt")
                nc.sync.dma_start(xt_t[:], xt_d[:, t * T:(t + 1) * T])
                # interleave the two MLPs at layer granularity so the PE
                # streams one MLP's matmuls while ACT tanh's the other's
                prev = [None, None]
                for l in range(4):
                    psb = [None, None]
                    for m in range(2):      # 0 = tb (global), 1 = br (local)
                        psl = [pspool.tile([128, T], F32, tag="ps",
                                           name=f"ps{l}_{t}_{m}_{jj}")
                               for jj in range(2)]
                        psb[m] = psl
                        if l == 0:
                            for j in range(2):
                                for cb in range(ncb):
                                    nc.tensor.matmul(
                                        psl[j][:, cb * 512:(cb + 1) * 512],
                                        w0_s[:, m * 256 + j * 128:
                                             m * 256 + (j + 1) * 128],
                                        xt_t[:, cb * 512:(cb + 1) * 512],
                                        start=True, stop=True)
                        else:
                            for j in range(2):
                                for k in range(2):
                                    b = ((m * 3 + (l - 1)) * 2 + k) * 2 + j
                                    for cb in range(ncb):
                                        nc.tensor.matmul(
                                            psl[j][:, cb * 512:(cb + 1) * 512],
                                            wk_s[:, b * 128:(b + 1) * 128],
                                            prev[m][:, k, cb * 512:(cb + 1) * 512],
                                            start=(k == 0), stop=(k == 1))
                    for m in range(2):
                        psl = psb[m]
                        if l < 3:
                            al = apool.tile([128, 2, T], DT, tag="a",
                                            name=f"a{l}_{t}_{m}")
                            for j in range(2):
                                col = m * 6 + l * 2 + j
                                nc.scalar.activation(
                                    al[:, j, :], psl[j][:], AF.Tanh,
                                    bias=bs_s[:, col:col + 1], scale=1.0)
                            prev[m] = al
                        else:
                            for j in range(2):
                                nc.vector.reduce_max(
                                    rm[:, m * 2 + j, t:t + 1], psl[j][:],
                                    axis=AX.X)
            for c in range(4):
                nc.vector.reduce_max(om_s[:, c:c + 1], rm[:, c, :], axis=AX.X)
            nc.sync.dma_start(om_d[:], om_s[:])
    nc.compile()
    return nc


def _get_nc():
    if "nc" not in _CACHE:
        _CACHE["nc"] = _build()
    return _CACHE["nc"]


def _pack_weights(g):
    """g maps name -> np.ndarray for the tb_*/br_* weights."""
    w0 = np.concatenate([g["tb_w0"], g["br_w0"]], axis=1).astype(NPDT)
    blocks = []
    for pre in ("tb", "br"):
        for l in (1, 2, 3):
            W = g[f"{pre}_w{l}"]
            for k in range(2):
                for j in range(2):
                    blocks.append(W[k * 128:(k + 1) * 128,
                                    j * 128:(j + 1) * 128])
    wk = np.ascontiguousarray(np.concatenate(blocks, axis=1), dtype=NPDT)
    bs = np.zeros((128, 12), np.float32)
    for mi, pre in enumerate(("tb", "br")):
        for l in range(3):
            bvec = g[f"{pre}_b{l}"]
            for j in range(2):
                bs[:, mi * 6 + l * 2 + j] = bvec[j * 128:(j + 1) * 128]
    return w0, wk, bs


def _run_device(x, g, trace=False):
    """Returns (tb_max, br_max) pre-bias maxima of shape (256,) each, plus
    the BassKernelResults (for profiling)."""
    w0, wk, bs = _pack_weights(g)
    in_maps = []
    for c in range(N_CORES):
        xt = np.ascontiguousarray(x[c * P:(c + 1) * P].T, dtype=NPDT)
        in_maps.append({"xt": xt, "w0": w0, "wk": wk, "bs": bs})
    res = run_bass_kernel_spmd(_get_nc(), in_maps, list(range(N_CORES)),
                               trace=trace)
    oms = np.stack([r["omax"] for r in res.results])     # (8, 128, 4)
    om = oms.max(axis=0)                                 # (128, 4)
    tb_max = np.concatenate([om[:, 0], om[:, 1]])        # (256,)
    br_max = np.concatenate([om[:, 2], om[:, 3]])
    return tb_max, br_max, res


def _mlp_np(h, layers):
    for w, b in layers[:-1]:
        h = np.tanh(h @ w + b)
    w, b = layers[-1]
    return h @ w + b


def kernel(x, y,
           tb_w0, tb_b0, tb_w1, tb_b1, tb_w2, tb_b2, tb_w3, tb_b3,
           br_w0, br_b0, br_w1, br_b1, br_w2, br_b2, br_w3, br_b3,
           tr_w0, tr_b0, tr_w1, tr_b1, tr_w2, tr_b2, tr_w3, tr_b3,
           o_w0, o_b0, o_w1, o_b1, o_w2, o_b2, _trace=False):
    x = np.asarray(x, np.float32)
    y = np.asarray(y, np.float32)
    g = {k: np.asarray(v, np.float32) for k, v in dict(
        tb_w0=tb_w0, tb_w1=tb_w1, tb_w2=tb_w2, tb_w3=tb_w3,
        br_w0=br_w0, br_w1=br_w1, br_w2=br_w2, br_w3=br_w3,
        tb_b0=tb_b0, tb_b1=tb_b1, tb_b2=tb_b2,
        br_b0=br_b0, br_b1=br_b1, br_b2=br_b2,
    ).items()}

    tb_pre, br_pre, res = _run_device(x, g, trace=_trace)
    _CACHE["last_results"] = res
    global_param = tb_pre + np.asarray(tb_b3, np.float32)   # (256,)
    local_param = br_pre + np.asarray(br_b3, np.float32)

    # patch gather (host): points whose bin id == PATCH_ID
    c = np.clip(np.floor(x * float(MNK)).astype(np.int64), 0, MNK - 1)
    pid = c[:, 0] * (MNK * MNK) + c[:, 1] * MNK + c[:, 2]
    idx = np.nonzero(pid == PATCH_ID)[0]
    x_patch = x[idx]
    gt_patch = y[idx]

    tr = [(np.asarray(tr_w0, np.float32), np.asarray(tr_b0, np.float32)),
          (np.asarray(tr_w1, np.float32), np.asarray(tr_b1, np.float32)),
          (np.asarray(tr_w2, np.float32), np.asarray(tr_b2, np.float32)),
          (np.asarray(tr_w3, np.float32), np.asarray(tr_b3, np.float32))]
    o = [(np.asarray(o_w0, np.float32), np.asarray(o_b0, np.float32)),
         (np.asarray(o_w1, np.float32), np.asarray(o_b1, np.float32)),
         (np.asarray(o_w2, np.float32), np.asarray(o_b2, np.float32))]

    local_coord = _mlp_np(x_patch, tr)                      # (MM, 256)
    mm = local_coord.shape[0]
    feat = np.concatenate([
        local_coord,
        np.broadcast_to(local_param, (mm, local_param.shape[0])),
        np.broadcast_to(global_param, (mm, global_param.shape[0])),
    ], axis=-1).astype(np.float32)
    pred_patch = _mlp_np(feat, o).astype(np.float32)
    return pred_patch, gt_patch


# revision 5
# speedup vs baseline: 1.9817x; 1.0003x over previous
"""Trainium2 Bass kernel for nn_DON_cnn_79216376807825 (histogram_binning).

Strategy (8 NeuronCores, data-parallel over points):
  - The dominant cost is two 4-layer MLPs (3->256->256->256->256, tanh) applied
    to all 262144 points, each followed by a max-reduction over points.
    Points are sharded 32768/core; each core computes its partial max of the
    final (pre-bias) layer output; host reduces over cores and adds the final
    bias (max(h@W + b) == max(h@W) + b).
  - On-chip layout: features on partitions, points on the free dim.  Weights
    are the stationary matmul operand (lhsT = W as stored, [K_in, M_out]);
    activations stream as the moving operand.  float32r matmuls (1 cyc/row,
    ~2e-4 rel err).  tanh+bias on the scalar engine reading PSUM, writing
    SBUF.  Final-layer PSUM is max-reduced on the vector engine.
  - The tiny patch part (gather of ~260 points in bin 995, tr-MLP, concat,
    o-MLP) runs on host in fp32 numpy - it is <0.03% of the FLOPs.
"""

import sys

if "/opt/trn_rl_repo" not in sys.path:
    sys.path.insert(0, "/opt/trn_rl_repo")

import numpy as np

import concourse.bass as bass  # noqa: F401  (engine registration side effects)
import concourse.mybir as mybir
from concourse import bacc, tile
from concourse.bass_utils import run_bass_kernel_spmd

N_CORES = 8
N_PTS = 262144
P = N_PTS // N_CORES          # 32768 points per core
T = 1024                      # points per macro-tile (psum tile = T f32 cols)
NT = P // T
H = 256
MNK = 10
PATCH_ID = 995

F32 = mybir.dt.float32
F32R = mybir.dt.float32r
F16 = mybir.dt.float16
DT = F16                      # matmul operand dtype (fp16: 1 cyc/row, ~4e-4)
NPDT = np.float16
AF = mybir.ActivationFunctionType
AX = mybir.AxisListType

_CACHE: dict = {}


def _build():
    nc = bacc.Bacc("TRN2", target_bir_lowering=False, debug=False,
                   num_devices=N_CORES)
    xt_d = nc.dram_tensor("xt", [3, P], DT, kind="ExternalInput").ap()
    w0_d = nc.dram_tensor("w0", [3, 512], DT, kind="ExternalInput").ap()
    wk_d = nc.dram_tensor("wk", [128, 3072], DT, kind="ExternalInput").ap()
    bs_d = nc.dram_tensor("bs", [128, 12], F32, kind="ExternalInput").ap()
    om_d = nc.dram_tensor("omax", [128, 4], F32, kind="ExternalOutput").ap()

    ncb = T // 512  # 512-col blocks per tile (matmul moving-operand limit)

    with tile.TileContext(nc) as tc:
        with tc.tile_pool(name="const", bufs=1) as cpool, \
             tc.tile_pool(name="xtp", bufs=6) as xpool, \
             tc.tile_pool(name="act", bufs=16) as apool, \
             tc.tile_pool(name="ps", bufs=4, space="PSUM") as pspool, \
             tc.tile_pool(name="red", bufs=1) as rpool:
            w0_s = cpool.tile([3, 512], DT, tag="w0")
            wk_s = cpool.tile([128, 3072], DT, tag="wk")
            bs_s = cpool.tile([128, 12], F32, tag="bs")
            nc.sync.dma_start(w0_s[:], w0_d[:])
            nc.sync.dma_start(wk_s[:], wk_d[:])
            nc.sync.dma_start(bs_s[:], bs_d[:])
            # per-(chunk, tile) reduced maxima; final pass reduces over tiles
            rm = rpool.tile([128, 4, NT], F32, tag="rm")
            om_s = rpool.tile([128, 4], F32, tag="om")

            for t in range(NT):
                xt_t = xpool.tile([3, T], DT, tag="xt")
                nc.sync.dma_start(xt_t[:], xt_d[:, t * T:(t + 1) * T])
                # interleave the two MLPs at layer granularity so the PE
                # streams one MLP's matmuls while ACT tanh's the other's
                prev = [None, None]
                for l in range(4):
                    psb = [None, None]
                    for m in range(2):      # 0 = tb (global), 1 = br (local)
                        psl = [pspool.tile([128, T], F32, tag="ps",
                                           name=f"ps{l}_{t}_{m}_{jj}")
                               for jj in range(2)]
                        psb[m] = psl
                        if l == 0:
                            for j in range(2):
                                for cb in range(ncb):
                                    nc.tensor.matmul(
                                        psl[j][:, cb * 512:(cb + 1) * 512],
                                        w0_s[:, m * 256 + j * 128:
                                             m * 256 + (j + 1) * 128],
                                        xt_t[:, cb * 512:(cb + 1) * 512],
                                        start=True, stop=True)
                        else:
                            for j in range(2):
                                for k in range(2):
                                    b = ((m * 3 + (l - 1)) * 2 + k) * 2 + j
                                    for cb in range(ncb):
                                        nc.tensor.matmul(
                                            psl[j][:, cb * 512:(cb + 1) * 512],
                                            wk_s[:, b * 128:(b + 1) * 128],
                                            prev[m][:, k, cb * 512:(cb + 1) * 512],
                                            start=(k == 0), stop=(k == 1))
                    for m in range(2):
                        psl = psb[m]
                        if l < 3:
                            al = apool.tile([128, 2, T], DT, tag="a",
                                            name=f"a{l}_{t}_{m}")
                            for j in range(2):
                                col = m * 6 + l * 2 + j
                                nc.scalar.activation(
                                    al[:, j, :], psl[j][:], AF.Tanh,
                                    bias=bs_s[:, col:col + 1], scale=1.0)
                            prev[m] = al
                        else:
                            for j in range(2):
                                nc.vector.reduce_max(
                                    rm[:, m * 2 + j, t:t + 1], psl[j][:],
                                    axis=AX.X)
            for c in range(4):
                nc.vector.reduce_max(om_s[:, c:c + 1], rm[:, c, :], axis=AX.X)
            nc.sync.dma_start(om_d[:], om_s[:])
    nc.compile()
    return nc


def _get_nc():
    if "nc" not in _CACHE:
        _CACHE["nc"] = _build()
    return _CACHE["nc"]


def _pack_weights(g):
    """g maps name -> np.ndarray for the tb_*/br_* weights."""
    w0 = np.concatenate([g["tb_w0"], g["br_w0"]], axis=1).astype(NPDT)
    blocks = []
    for pre in ("tb", "br"):
        for l in (1, 2, 3):
            W = g[f"{pre}_w{l}"]
            for k in range(2):
                for j in range(2):
                    blocks.append(W[k * 128:(k + 1) * 128,
                                    j * 128:(j + 1) * 128])
    wk = np.ascontiguousarray(np.concatenate(blocks, axis=1), dtype=NPDT)
    bs = np.zeros((128, 12), np.float32)
    for mi, pre in enumerate(("tb", "br")):
        for l in range(3):
            bvec = g[f"{pre}_b{l}"]
            for j in range(2):
                bs[:, mi * 6 + l * 2 + j] = bvec[j * 128:(j + 1) * 128]
    return w0, wk, bs


def _run_device(x, g, trace=False):
    """Returns (tb_max, br_max) pre-bias maxima of shape (256,) each, plus
    the BassKernelResults (for profiling)."""
    w0, wk, bs = _pack_weights(g)
    in_maps = []
    for c in range(N_CORES):
        xt = np.ascontiguousarray(x[c * P:(c + 1) * P].T, dtype=NPDT)
        in_maps.append({"xt": xt, "w0": w0, "wk": wk, "bs": bs})
    res = run_bass_kernel_spmd(_get_nc(), in_maps, list(range(N_CORES)),
                               trace=trace)
    oms = np.stack([r["omax"] for r in res.results])     # (8, 128, 4)
    om = oms.max(axis=0)                                 # (128, 4)
    tb_max = np.concatenate([om[:, 0], om[:, 1]])        # (256,)
    br_max = np.concatenate([om[:, 2], om[:, 3]])
    return tb_max, br_max, res


def _mlp_np(h, layers):
    for w, b in layers[:-1]:
        h = np.tanh(h @ w + b)
    w, b = layers[-1]
    return h @ w + b


def kernel(x, y,
           tb_w0, tb_b0, tb_w1, tb_b1, tb_w2, tb_b2, tb_w3, tb_b3,
           br_w0, br_b0, br_w1, br_b1, br_w2, br_b2, br_w3, br_b3,
           tr_w0, tr_b0, tr_w1, tr_b1, tr_w2, tr_b2, tr_w3, tr_b3,
           o_w0, o_b0, o_w1, o_b1, o_w2, o_b2, _trace=False):
    x = np.asarray(x, np.float32)
    y = np.asarray(y, np.float32)
    g = {k: np.asarray(v, np.float32) for k, v in dict(
        tb_w0=tb_w0, tb_w1=tb_w1, tb_w2=tb_w2, tb_w3=tb_w3,
        br_w0=br_w0, br_w1=br_w1, br_w2=br_w2, br_w3=br_w3,
        tb_b0=tb_b0, tb_b1=tb_b1, tb_b2=tb_b2,
        br_b0=br_b0, br_b1=br_b1, br_b2=br_b2,
    ).items()}

    tb_pre, br_pre, res = _run_device(x, g, trace=_trace)
    _CACHE["last_results"] = res
    global_param = tb_pre + np.asarray(tb_b3, np.float32)   # (256,)
    local_param = br_pre + np.asarray(br_b3, np.float32)

    # patch gather (host): points whose bin id == PATCH_ID
    c = np.clip(np.floor(x * float(MNK)).astype(np.int64), 0, MNK - 1)
    pid = c[:, 0] * (MNK * MNK) + c[:, 1] * MNK + c[:, 2]
    idx = np.nonzero(pid == PATCH_ID)[0]
    x_patch = x[idx]
    gt_patch = y[idx]

    tr = [(np.asarray(tr_w0, np.float32), np.asarray(tr_b0, np.float32)),
          (np.asarray(tr_w1, np.float32), np.asarray(tr_b1, np.float32)),
          (np.asarray(tr_w2, np.float32), np.asarray(tr_b2, np.float32)),
          (np.asarray(tr_w3, np.float32), np.asarray(tr_b3, np.float32))]
    o = [(np.asarray(o_w0, np.float32), np.asarray(o_b0, np.float32)),
         (np.asarray(o_w1, np.float32), np.asarray(o_b1, np.float32)),
         (np.asarray(o_w2, np.float32), np.asarray(o_b2, np.float32))]

    local_coord = _mlp_np(x_patch, tr)                      # (MM, 256)
    mm = local_coord.shape[0]
    feat = np.concatenate([
        local_coord,
        np.broadcast_to(local_param, (mm, local_param.shape[0])),
        np.broadcast_to(global_param, (mm, global_param.shape[0])),
    ], axis=-1).astype(np.float32)
    pred_patch = _mlp_np(feat, o).astype(np.float32)
    return pred_patch, gt_patch
